# revision 7
# baseline (speedup 1.0000x reference)
"""Trainium2 Bass kernel for MiniTriangularUpdate (B=1, N=768, D=128) on 8 cores.

Sharding: phase P (in-LN + gated in-projection) row-sharded (96 rows/core);
AllToAll #1 re-shards the gated pair activations by channel; phase E computes
the two triangular einsums channel-sharded (8 channels/core, each a batch of
768x768x768 matmuls); AllToAll #2 re-shards the pair output back to rows;
phase O (out-LN + gated out-projection) row-sharded.

LayerNorm scale/bias are folded into the projection weights on the host
(W' = diag(w) @ W, c = b @ W), so the device only computes z = (x-mean)*rstd.
"""

import sys

sys.path.insert(0, "/opt/trn_rl_repo")

import numpy as np
import ml_dtypes
from contextlib import ExitStack

import concourse.bass as bass
import concourse.tile as tile
import concourse.mybir as mybir
from concourse import bass_utils
from concourse.masks import make_identity

# ---- problem constants (hardcoded per contract) ----
N = 768          # sequence length (both pair dims)
D = 128          # input channels
DH = 64          # pair channels after split (x1|x2)
NC = 8           # cores
RB = N // NC     # rows per core = 96
TPC = 4          # channels of each of a1/b1/a2/b2 per core (32/8)
CPD = 16         # channels per dest in the A2A#1 chunk (4*TPC)
EPS = 1e-5

F32 = mybir.dt.float32
BF16 = mybir.dt.bfloat16

# transport dtype for the big intermediates (A2A payloads + matmul operands)
TDT = BF16
TDT_NP = ml_dtypes.bfloat16


def _split_multiwaits(nc):
    """The TRN2 ISA has one wait slot per instruction and this walrus build
    refuses to legalize multi-wait instructions — split the extras onto
    same-engine NOPs inserted right before the offending instruction."""
    for f in nc.m.functions:
        for bb in f.blocks:
            insts = list(bb.instructions)
            out = []
            changed = False
            for ins in insts:
                si = ins.sync_info
                if si is not None and si.on_wait and len(si.on_wait) > 1:
                    waits = list(si.on_wait)
                    for k, w in enumerate(waits[:-1]):
                        out.append(mybir.InstNoOp(
                            name=f"{ins.name}-wsplit{k}",
                            engine=ins.engine,
                            ins=[], outs=[],
                            sync_info=mybir.SyncInfo(on_wait=[w], on_update=[]),
                        ))
                    ins.sync_info = mybir.SyncInfo(
                        on_wait=[waits[-1]], on_update=list(si.on_update or []))
                    changed = True
                out.append(ins)
            if changed:
                try:
                    bb.instructions = out
                except Exception:
                    bb.instructions.clear()
                    for i in out:
                        bb.add_instruction(i)


def build_kernel(c1_nonzero: bool, c2_nonzero: bool):
    """Build the SPMD Bass program (identical on all 8 cores)."""
    nc = bass.Bass(
        "TRN2", target_bir_lowering=False, debug=False, num_devices=NC
    )

    # ---------------- kernel I/O ----------------
    x_in = nc.dram_tensor("x_rows", [RB, N, D], TDT, kind="ExternalInput").ap()
    mask_in = nc.dram_tensor("mask_rows", [RB, N], TDT, kind="ExternalInput").ap()
    # W1: (D, 256) = [diag(w)Wp | diag(w)Wg], columns permuted dest-major so
    # partition block d*16:(d+1)*16 of the projection output holds dest d's
    # 16 channels [a1(4) b1(4) a2(4) b2(4)].
    w1_in = nc.dram_tensor("w1", [D, 2 * D], TDT, kind="ExternalInput").ap()
    # W2: (64, 256) = [W_p_out' | W_g_out'] with rows permuted to the
    # assembled (src, t') channel order of the A2A#2 output.
    w2_in = nc.dram_tensor("w2", [DH, 2 * D], TDT, kind="ExternalInput").ap()
    c1_in = nc.dram_tensor("c1", [1, 2 * D], TDT, kind="ExternalInput").ap()
    c2_in = nc.dram_tensor("c2", [1, 2 * D], TDT, kind="ExternalInput").ap()
    out = nc.dram_tensor("out_rows", [RB, N, D], F32, kind="ExternalOutput").ap()

    rg = [list(range(NC))]
    NT = N // D  # 6 tiles of 128 along a 768 axis

    with tile.TileContext(nc) as tc, ExitStack() as ctx:
        # ---------------- DRAM intermediates ----------------
        dram = ctx.enter_context(tc.tile_pool(name="dram", bufs=1, space="DRAM"))
        # A2A#1: [dest, r, t'(16), c] ; t' = [a1 x4, b1 x4, a2 x4, b2 x4]
        a2a1_in = dram.tile([NC, RB, CPD, N], TDT)
        a2a1_out = dram.tile([NC, RB, CPD, N], TDT)
        # A2A#2: [dest, tt(8), i(96), j] ; tt = [x1 ch x4, x2 ch x4]
        a2a2_in = dram.tile([NC, 8, RB, N], TDT)
        a2a2_out = dram.tile([NC, 8, RB, N], TDT)

        # ---------------- SBUF constants ----------------
        const = ctx.enter_context(tc.tile_pool(name="const", bufs=1))
        ident = const.tile([D, D], TDT)
        make_identity(nc, ident)
        w1_sb = const.tile([D, 2 * D], TDT)
        nc.sync.dma_start(out=w1_sb, in_=w1_in)
        w2_sb = const.tile([DH, 2 * D], TDT)
        nc.sync.dma_start(out=w2_sb, in_=w2_in)
        eps_sb = const.tile([D, 1], F32)
        nc.vector.memset(eps_sb, EPS)
        ones_sb = const.tile([1, 512], TDT)
        nc.vector.memset(ones_sb, 1.0)
        c1_sb = const.tile([1, 2 * D], TDT)
        if c1_nonzero:
            nc.sync.dma_start(out=c1_sb, in_=c1_in)
        c2_sb = const.tile([1, 2 * D], TDT)
        if c2_nonzero:
            nc.sync.dma_start(out=c2_sb, in_=c2_in)

        # =========================================================
        # Phase P: per local row r: LN + gated projection + mask
        # =========================================================
        with ExitStack() as pctx:
            p_x = pctx.enter_context(tc.tile_pool(name="p_x", bufs=3))
            p_tmp = pctx.enter_context(tc.tile_pool(name="p_tmp", bufs=4))
            p_stat = pctx.enter_context(tc.tile_pool(name="p_stat", bufs=8))
            p_zt = pctx.enter_context(tc.tile_pool(name="p_zt", bufs=2))
            p_hh = pctx.enter_context(tc.tile_pool(name="p_hh", bufs=2))
            ps_t = pctx.enter_context(tc.tile_pool(name="ps_t", bufs=2, space="PSUM"))
            ps_p = pctx.enter_context(tc.tile_pool(name="ps_p", bufs=2, space="PSUM"))
            ps_g = pctx.enter_context(tc.tile_pool(name="ps_g", bufs=2, space="PSUM"))
            p_msk = pctx.enter_context(tc.tile_pool(name="p_msk", bufs=2))

            for r in range(RB):
                # load x[r] as 6 natural tiles (c-part, din-free)
                xt = p_x.tile([D, NT, D], TDT, tag="xt")
                nc.sync.dma_start(
                    out=xt, in_=x_in[r].rearrange("(t p) d -> p t d", p=D)
                )
                zt_slab = p_zt.tile([D, N], TDT, tag="zt")
                # mask[r] broadcast to all 128 partitions (stride-0 DMA)
                mask_bc = p_msk.tile([D, N], TDT, tag="mbc")
                nc.sync.dma_start(
                    out=mask_bc,
                    in_=bass.AP(
                        tensor=mask_in.tensor,
                        offset=mask_in.offset + r * N,
                        ap=[[0, D], [1, N]],
                    ),
                )
                for t in range(NT):
                    # stats per token (tokens on partitions)
                    st = p_stat.tile([D, 6], F32, tag="st")
                    nc.vector.bn_stats(out=st, in_=xt[:, t, :])
                    mv = p_stat.tile([D, 2], F32, tag="mv")
                    nc.vector.bn_aggr(out=mv, in_=st)
                    rstd = p_stat.tile([D, 1], F32, tag="rstd")
                    nc.scalar.activation(
                        out=rstd, in_=mv[:, 1:2],
                        func=mybir.ActivationFunctionType.Sqrt,
                        bias=eps_sb, scale=1.0,
                    )
                    nc.vector.reciprocal(out=rstd, in_=rstd)
                    # z = (x - mean) * rstd   (bf16 out)
                    z = p_tmp.tile([D, D], TDT, tag="z")
                    nc.gpsimd.tensor_scalar(
                        out=z, in0=xt[:, t, :],
                        scalar1=mv[:, 0:1], scalar2=rstd,
                        op0=mybir.AluOpType.subtract, op1=mybir.AluOpType.mult,
                    )
                    # transpose z -> (din, c) into zt_slab
                    zpt = ps_t.tile([D, D], TDT, tag="zpt")
                    nc.tensor.transpose(zpt, z, ident)
                    nc.vector.tensor_copy(out=zt_slab[:, t * D:(t + 1) * D], in_=zpt)

                # gated projection, two 384-wide chunks over c
                hh = p_hh.tile([D, N], TDT, tag="hh")
                CH = 384
                for ch in range(N // CH):
                    cs = slice(ch * CH, (ch + 1) * CH)
                    pp = ps_p.tile([D, CH], F32, tag="pp")
                    pg = ps_g.tile([D, CH], F32, tag="pg")
                    nc.tensor.matmul(pp, w1_sb[:, 0:D], zt_slab[:, cs],
                                     start=True, stop=not c1_nonzero)
                    nc.tensor.matmul(pg, w1_sb[:, D:2 * D], zt_slab[:, cs],
                                     start=True, stop=not c1_nonzero)
                    if c1_nonzero:
                        nc.tensor.matmul(pp, c1_sb[:, 0:D], ones_sb[:, :CH],
                                         start=False, stop=True,
                                         skip_group_check=True)
                        nc.tensor.matmul(pg, c1_sb[:, D:2 * D], ones_sb[:, :CH],
                                         start=False, stop=True,
                                         skip_group_check=True)
                    # s = sigmoid(g) ; t1 = p*s ; hh = t1*mask
                    s = p_tmp.tile([D, CH], F32, tag="s")
                    nc.scalar.activation(
                        out=s, in_=pg, func=mybir.ActivationFunctionType.Sigmoid,
                    )
                    t1 = p_tmp.tile([D, CH], F32, tag="t1")
                    nc.vector.tensor_tensor(t1, pp, s, mybir.AluOpType.mult)
                    nc.gpsimd.tensor_tensor(hh[:, cs], t1, mask_bc[:, cs],
                                            mybir.AluOpType.mult)

                # ship: partition block d*16.. holds dest d's channels
                for d in range(NC):
                    nc.sync.dma_start(
                        out=a2a1_in[d, r],
                        in_=hh[d * CPD:(d + 1) * CPD, :],
                    )

        # =========================================================
        # AllToAll #1
        # =========================================================
        nc.gpsimd.collective_compute(
            "AllToAll", mybir.AluOpType.bypass,
            replica_groups=rg,
            ins=[a2a1_in[:].opt()],
            outs=[a2a1_out[:].opt()],
        )

        # =========================================================
        # Phase E: einsums, channel-sharded.
        # a2a1_out[src, r, t', c]: global row = src*96+r.
        #   t' 0:4 a1, 4:8 b1 (need transpose to (c, row));
        #   t' 8:12 a2, 12:16 b2 (natural (row, c) is already (k, free)).
        # =========================================================
        with ExitStack() as ectx:
            e_nat = ectx.enter_context(tc.tile_pool(name="e_nat", bufs=2))
            e_op = ectx.enter_context(tc.tile_pool(name="e_op", bufs=2))
            e_st = ectx.enter_context(tc.tile_pool(name="e_st", bufs=3))
            ps_e = ectx.enter_context(
                tc.tile_pool(name="ps_e", bufs=3, space="PSUM"))
            ps_et = ectx.enter_context(
                tc.tile_pool(name="ps_et", bufs=2, space="PSUM"))

            def load_nat(tp):
                """Load channel tp as natural (row, c) slab (128, 6, 768)."""
                slab = e_nat.tile([D, NT, N], TDT, tag="nat")
                for src in range(NC):
                    r0 = src * RB  # global row of piece start
                    lo = r0
                    while lo < r0 + RB:
                        kt = lo // D
                        hi = min(r0 + RB, (kt + 1) * D)
                        nc.sync.dma_start(
                            out=slab[lo - kt * D: hi - kt * D, kt, :],
                            in_=a2a1_out[src, lo - r0: hi - r0, tp, :],
                        )
                        lo = hi
                return slab

            def transpose_slab(nat):
                """(row, c) natural slab -> (c, row) slab, both (128, 6, 768)."""
                tsl = e_op.tile([D, NT, N], TDT, tag="top")
                for rt in range(NT):
                    for ct in range(NT):
                        pt = ps_et.tile([D, D], TDT, tag="pt")
                        nc.tensor.transpose(
                            pt, nat[:, rt, ct * D:(ct + 1) * D], ident)
                        nc.vector.tensor_copy(
                            out=tsl[:, ct, rt * D:(rt + 1) * D], in_=pt)
                return tsl

            JC = 384
            for tt in range(2 * TPC):  # 4 x1-channels then 4 x2-channels
                if tt < TPC:
                    lhs_nat = load_nat(tt)          # a1 (i,k) natural
                    lhsT = transpose_slab(lhs_nat)  # (k, i)
                    rhs_nat = load_nat(TPC + tt)    # b1 (j,k) natural
                    rhsT = transpose_slab(rhs_nat)  # (k, j)
                else:
                    lhsT = load_nat(2 * TPC + (tt - TPC))  # a2 (k,i) natural
                    rhsT = load_nat(3 * TPC + (tt - TPC))  # b2 (k,j) natural
                for it in range(NT):
                    for jc in range(N // JC):
                        pe = ps_e.tile([D, JC], F32, tag="pe")
                        for kt in range(NT):
                            nc.tensor.matmul(
                                pe,
                                lhsT[:, kt, it * D:(it + 1) * D],
                                rhsT[:, kt, jc * JC:(jc + 1) * JC],
                                start=(kt == 0), stop=(kt == NT - 1),
                            )
                        st = e_st.tile([D, JC], TDT, tag="est")
                        nc.vector.tensor_copy(out=st, in_=pe)
                        # write to a2a2_in[dest, tt, i_local, j]
                        lo = it * D
                        while lo < (it + 1) * D:
                            dst = lo // RB
                            hi = min((it + 1) * D, (dst + 1) * RB)
                            nc.sync.dma_start(
                                out=a2a2_in[dst, tt, lo - dst * RB: hi - dst * RB,
                                            jc * JC:(jc + 1) * JC],
                                in_=st[lo - it * D: hi - it * D, :],
                            )
                            lo = hi

        # =========================================================
        # AllToAll #2
        # =========================================================
        nc.gpsimd.collective_compute(
            "AllToAll", mybir.AluOpType.bypass,
            replica_groups=rg,
            ins=[a2a2_in[:].opt()],
            outs=[a2a2_out[:].opt()],
        )

        # =========================================================
        # Phase O: out-LN + gated out-projection, row-sharded.
        # a2a2_out[src, tt, i, j] -> x12T (64ch, 768) per local row i.
        # =========================================================
        with ExitStack() as octx:
            o_xt = octx.enter_context(tc.tile_pool(name="o_xt", bufs=3))
            o_tmp = octx.enter_context(tc.tile_pool(name="o_tmp", bufs=4))
            o_stat = octx.enter_context(tc.tile_pool(name="o_stat", bufs=8))
            o_zt = octx.enter_context(tc.tile_pool(name="o_zt", bufs=2))
            o_out = octx.enter_context(tc.tile_pool(name="o_out", bufs=2))
            ps_oti = octx.enter_context(
                tc.tile_pool(name="ps_oti", bufs=2, space="PSUM"))
            ps_oto = octx.enter_context(
                tc.tile_pool(name="ps_oto", bufs=2, space="PSUM"))
            ps_o = octx.enter_context(
                tc.tile_pool(name="ps_o", bufs=2, space="PSUM"))

            for i in range(RB):
                x12t = o_xt.tile([DH, N], TDT, tag="x12t")
                nc.sync.dma_start(
                    out=x12t,
                    in_=a2a2_out[:, :, i, :].rearrange("s t j -> (s t) j"),
                )
                z2t_slab = o_zt.tile([DH, N], TDT, tag="z2t")
                outsl = o_out.tile([D, NT, D], F32, tag="outsl")
                for t in range(NT):
                    ts_ = slice(t * D, (t + 1) * D)
                    # transpose to natural (tok, ch)
                    pnat = ps_oti.tile([D, DH], TDT, tag="pnat")
                    nc.tensor.transpose(pnat, x12t[:, ts_], ident[:DH, :DH])
                    st = o_stat.tile([D, 6], F32, tag="ost")
                    nc.vector.bn_stats(out=st, in_=pnat)
                    mv = o_stat.tile([D, 2], F32, tag="omv")
                    nc.vector.bn_aggr(out=mv, in_=st)
                    rstd = o_stat.tile([D, 1], F32, tag="orstd")
                    nc.scalar.activation(
                        out=rstd, in_=mv[:, 1:2],
                        func=mybir.ActivationFunctionType.Sqrt,
                        bias=eps_sb, scale=1.0,
                    )
                    nc.vector.reciprocal(out=rstd, in_=rstd)
                    z2 = o_tmp.tile([D, DH], TDT, tag="z2")
                    nc.vector.tensor_scalar(
                        out=z2, in0=pnat,
                        scalar1=mv[:, 0:1], scalar2=rstd,
                        op0=mybir.AluOpType.subtract, op1=mybir.AluOpType.mult,
                    )
                    # transpose back to (ch, tok)
                    pzt = ps_oto.tile([DH, D], TDT, tag="pzt")
                    nc.tensor.transpose(pzt, z2, ident)
                    nc.vector.tensor_copy(out=z2t_slab[:, ts_], in_=pzt)

                    # gated projection: psum (128 tok, 256)
                    po = ps_o.tile([D, 2 * D], F32, tag="po")
                    nc.tensor.matmul(po, z2t_slab[:, ts_], w2_sb,
                                     start=True, stop=not c2_nonzero)
                    if c2_nonzero:
                        nc.tensor.matmul(po, ones_sb[:, 0:D], c2_sb,
                                         start=False, stop=True,
                                         skip_group_check=True)
                    s2 = o_tmp.tile([D, D], F32, tag="s2")
                    nc.scalar.activation(
                        out=s2, in_=po[:, D:2 * D],
                        func=mybir.ActivationFunctionType.Sigmoid,
                    )
                    nc.vector.tensor_tensor(
                        outsl[:, t, :], po[:, 0:D], s2, mybir.AluOpType.mult
                    )
                nc.sync.dma_start(
                    out=out[i].rearrange("(t p) d -> p t d", p=D),
                    in_=outsl,
                )

    _split_multiwaits(nc)
    return nc


_BUILD_CACHE = {}


def kernel(x, mask, w_norm_in, b_norm_in, W_p_in, W_g_in,
           w_norm_out, b_norm_out, W_p_out, W_g_out):
    x = np.asarray(x, np.float32)
    mask = np.asarray(mask, np.float32)
    w_norm_in = np.asarray(w_norm_in, np.float32)
    b_norm_in = np.asarray(b_norm_in, np.float32)
    W_p_in = np.asarray(W_p_in, np.float32)
    W_g_in = np.asarray(W_g_in, np.float32)
    w_norm_out = np.asarray(w_norm_out, np.float32)
    b_norm_out = np.asarray(b_norm_out, np.float32)
    W_p_out = np.asarray(W_p_out, np.float32)
    W_g_out = np.asarray(W_g_out, np.float32)

    B = x.shape[0]
    assert x.shape == (B, N, N, D) and B == 1

    # ---- host-side weight folding & permutations ----
    W1p = w_norm_in[:, None] * W_p_in
    W1g = w_norm_in[:, None] * W_g_in
    c1 = np.concatenate([b_norm_in @ W_p_in, b_norm_in @ W_g_in])
    # dest-major output-channel permutation for phase P. hh channel c uses
    # p-col c and g-col c, so permute both the same way.
    perm = np.empty(D, np.int64)
    for d in range(NC):
        blk = d * CPD
        perm[blk:blk + 4] = np.arange(4) + 4 * d             # a1
        perm[blk + 4:blk + 8] = 32 + np.arange(4) + 4 * d    # b1
        perm[blk + 8:blk + 12] = 64 + np.arange(4) + 4 * d   # a2
        perm[blk + 12:blk + 16] = 96 + np.arange(4) + 4 * d  # b2
    W1 = np.concatenate([W1p[:, perm], W1g[:, perm]], axis=1)
    c1_perm = np.concatenate([c1[:D][perm], c1[D:][perm]])[None, :]

    # phase O: assembled channel q = src*8 + tt ; global channel:
    #   tt<4 -> 4*src+tt (x1 half), tt>=4 -> 32 + 4*src + (tt-4) (x2 half)
    qperm = np.empty(DH, np.int64)
    for src in range(NC):
        for t2 in range(8):
            g = 4 * src + t2 if t2 < 4 else 32 + 4 * src + (t2 - 4)
            qperm[src * 8 + t2] = g
    W2p = w_norm_out[:, None] * W_p_out
    W2g = w_norm_out[:, None] * W_g_out
    W2 = np.concatenate([W2p[qperm, :], W2g[qperm, :]], axis=1)
    c2 = np.concatenate([b_norm_out @ W_p_out, b_norm_out @ W_g_out])[None, :]

    c1_nz = bool(np.any(c1_perm != 0))
    c2_nz = bool(np.any(c2 != 0))

    key = (c1_nz, c2_nz)
    if key not in _BUILD_CACHE:
        _BUILD_CACHE[key] = build_kernel(c1_nz, c2_nz)
    nc_prog = _BUILD_CACHE[key]

    xs = x[0].astype(TDT_NP)                  # (768, 768, 128)
    w1_np = np.ascontiguousarray(W1.astype(TDT_NP))
    w2_np = np.ascontiguousarray(W2.astype(TDT_NP))
    in_maps = []
    for c in range(NC):
        in_maps.append({
            "x_rows": np.ascontiguousarray(xs[c * RB:(c + 1) * RB]),
            "mask_rows": np.ascontiguousarray(
                mask[0, c * RB:(c + 1) * RB].astype(TDT_NP)),
            "w1": w1_np,
            "w2": w2_np,
            "c1": c1_perm.astype(TDT_NP),
            "c2": c2.astype(TDT_NP),
        })

    res = bass_utils.run_bass_kernel_spmd(
        nc_prog, in_maps, core_ids=list(range(NC)),
        **getattr(kernel, "_run_kwargs", {}),
    )
    kernel._last_results = res
    outp = np.concatenate(
        [res.results[c]["out_rows"] for c in range(NC)], axis=0)
    return outp[None]  # (1, 768, 768, 128)


if __name__ == "__main__":
    import reference as R
    inputs = {k: np.asarray(v) for k, v in R.setup_inputs().items()}
    got = kernel(**inputs)
    exp = np.asarray(R.reference(**inputs))
    err = np.abs(got - exp)
    print("max abs err", err.max(), "absmax rel", err.max() / np.abs(exp).max())


# revision 11
# speedup vs baseline: 1.5669x; 1.5669x over previous
"""Trainium2 Bass kernel for MiniTriangularUpdate (B=1, N=768, D=128) on 8 cores.

Sharding: phase P (in-LN + gated in-projection) row-sharded (96 rows/core);
AllToAll #1 re-shards the gated pair activations by channel; phase E computes
the two triangular einsums channel-sharded (8 channels/core, each a batch of
768x768x768 matmuls); AllToAll #2 re-shards the pair output back to rows;
phase O (out-LN + gated out-projection) row-sharded.

LayerNorm scale/bias are folded into the projection weights on the host
(W' = diag(w) @ W, c = b @ W), so the device only computes z = (x-mean)*rstd.
"""

import sys

sys.path.insert(0, "/opt/trn_rl_repo")

import numpy as np
import ml_dtypes
from contextlib import ExitStack

import concourse.bass as bass
import concourse.tile as tile
import concourse.mybir as mybir
from concourse import bass_utils
from concourse.masks import make_identity

# ---- problem constants (hardcoded per contract) ----
N = 768          # sequence length (both pair dims)
D = 128          # input channels
DH = 64          # pair channels after split (x1|x2)
NC = 8           # cores
RB = N // NC     # rows per core = 96
TPC = 4          # channels of each of a1/b1/a2/b2 per core (32/8)
CPD = 16         # channels per dest in the A2A#1 chunk (4*TPC)
EPS = 1e-5

F32 = mybir.dt.float32
BF16 = mybir.dt.bfloat16

# transport dtype for the big intermediates (A2A payloads + matmul operands)
TDT = BF16
TDT_NP = ml_dtypes.bfloat16


def _split_multiwaits(nc):
    """The TRN2 ISA has one wait slot per instruction and this walrus build
    refuses to legalize multi-wait instructions — split the extras onto
    same-engine NOPs inserted right before the offending instruction."""
    for f in nc.m.functions:
        for bb in f.blocks:
            insts = list(bb.instructions)
            out = []
            changed = False
            for ins in insts:
                si = ins.sync_info
                if si is not None and si.on_wait and len(si.on_wait) > 1:
                    waits = list(si.on_wait)
                    for k, w in enumerate(waits[:-1]):
                        out.append(mybir.InstNoOp(
                            name=f"{ins.name}-wsplit{k}",
                            engine=ins.engine,
                            ins=[], outs=[],
                            sync_info=mybir.SyncInfo(on_wait=[w], on_update=[]),
                        ))
                    ins.sync_info = mybir.SyncInfo(
                        on_wait=[waits[-1]], on_update=list(si.on_update or []))
                    changed = True
                out.append(ins)
            if changed:
                try:
                    bb.instructions = out
                except Exception:
                    bb.instructions.clear()
                    for i in out:
                        bb.add_instruction(i)


def build_kernel(c1_nonzero: bool, c2_nonzero: bool):
    """Build the SPMD Bass program (identical on all 8 cores)."""
    nc = bass.Bass(
        "TRN2", target_bir_lowering=False, debug=False, num_devices=NC
    )

    # ---------------- kernel I/O ----------------
    x_in = nc.dram_tensor("x_rows", [RB, N, D], TDT, kind="ExternalInput").ap()
    mask_in = nc.dram_tensor("mask_rows", [RB, N], TDT, kind="ExternalInput").ap()
    # W1: (D, 256) = [diag(w)Wp | diag(w)Wg], columns permuted dest-major so
    # partition block d*16:(d+1)*16 of the projection output holds dest d's
    # 16 channels [a1(4) b1(4) a2(4) b2(4)].
    w1_in = nc.dram_tensor("w1", [D, 2 * D], TDT, kind="ExternalInput").ap()
    # W2: (64, 256) = [W_p_out' | W_g_out'] with rows permuted to the
    # assembled (src, t') channel order of the A2A#2 output.
    w2_in = nc.dram_tensor("w2", [DH, 2 * D], TDT, kind="ExternalInput").ap()
    c1_in = nc.dram_tensor("c1", [1, 2 * D], TDT, kind="ExternalInput").ap()
    c2_in = nc.dram_tensor("c2", [1, 2 * D], TDT, kind="ExternalInput").ap()
    out = nc.dram_tensor("out_rows", [RB, N, D], F32, kind="ExternalOutput").ap()

    rg = [list(range(NC))]
    NT = N // D  # 6 tiles of 128 along a 768 axis

    with tile.TileContext(nc) as tc, ExitStack() as ctx:
        # ---------------- DRAM intermediates ----------------
        dram = ctx.enter_context(tc.tile_pool(name="dram", bufs=1, space="DRAM"))
        # A2A#1: [dest, r, t'(16), c] ; t' = [a1 x4, b1 x4, a2 x4, b2 x4]
        a2a1_in = dram.tile([NC, RB, CPD, N], TDT)
        a2a1_out = dram.tile([NC, RB, CPD, N], TDT)
        # A2A#2: [dest, tt(8), i(96), j] ; tt = [x1 ch x4, x2 ch x4]
        a2a2_in = dram.tile([NC, 8, RB, N], TDT)
        a2a2_out = dram.tile([NC, 8, RB, N], TDT)

        # ---------------- SBUF constants ----------------
        const = ctx.enter_context(tc.tile_pool(name="const", bufs=1))
        ident = const.tile([D, D], TDT)
        make_identity(nc, ident)
        w1_sb = const.tile([D, 2 * D], TDT)
        nc.sync.dma_start(out=w1_sb, in_=w1_in)
        w2_sb = const.tile([DH, 2 * D], TDT)
        nc.sync.dma_start(out=w2_sb, in_=w2_in)
        eps_sb = const.tile([D, 1], F32)
        nc.vector.memset(eps_sb, EPS)
        ones_sb = const.tile([1, 512], TDT)
        nc.vector.memset(ones_sb, 1.0)
        c1_sb = const.tile([1, 2 * D], TDT)
        if c1_nonzero:
            nc.sync.dma_start(out=c1_sb, in_=c1_in)
        c2_sb = const.tile([1, 2 * D], TDT)
        if c2_nonzero:
            nc.sync.dma_start(out=c2_sb, in_=c2_in)

        # =========================================================
        # Phase P: per local row r: LN + gated projection + mask
        # =========================================================
        I32 = mybir.dt.int32
        with ExitStack() as pctx:
            p_x = pctx.enter_context(tc.tile_pool(name="p_x", bufs=3))
            p_tmp = pctx.enter_context(tc.tile_pool(name="p_tmp", bufs=4))
            p_stat = pctx.enter_context(tc.tile_pool(name="p_stat", bufs=6))
            p_zt = pctx.enter_context(tc.tile_pool(name="p_zt", bufs=2))
            p_hh = pctx.enter_context(tc.tile_pool(name="p_hh", bufs=2))
            p_msk = pctx.enter_context(tc.tile_pool(name="p_msk", bufs=2))
            ps_t = pctx.enter_context(tc.tile_pool(name="ps_t", bufs=2, space="PSUM"))
            ps_p = pctx.enter_context(tc.tile_pool(name="ps_p", bufs=2, space="PSUM"))
            ps_g = pctx.enter_context(tc.tile_pool(name="ps_g", bufs=2, space="PSUM"))

            def newton_rsqrt(pool, var_view, P):
                """rstd = 1/sqrt(var_view + EPS) via DVE-only bit-trick Newton.
                var_view: (P, G) f32 view. Returns (P, G) f32 tile."""
                G = var_view.shape[-1]
                veps = pool.tile([P, G], F32, tag="nr_veps")
                nc.vector.tensor_scalar(
                    out=veps, in0=var_view, scalar1=EPS, scalar2=None,
                    op0=mybir.AluOpType.add)
                y = pool.tile([P, G], F32, tag="nr_y")
                tsh = pool.tile([P, G], I32, tag="nr_tsh")
                nc.vector.tensor_scalar(
                    out=tsh, in0=veps.bitcast(I32), scalar1=1, scalar2=None,
                    op0=mybir.AluOpType.logical_shift_right)
                nc.vector.tensor_scalar(
                    out=y.bitcast(I32), in0=tsh, scalar1=-1,
                    scalar2=0x5F3759DF,
                    op0=mybir.AluOpType.mult, op1=mybir.AluOpType.add)
                t2 = pool.tile([P, G], F32, tag="nr_t2")
                for _ in range(2):
                    nc.vector.tensor_tensor(t2, y, y, mybir.AluOpType.mult)
                    nc.vector.tensor_tensor(t2, t2, veps, mybir.AluOpType.mult)
                    nc.vector.tensor_scalar(
                        out=t2, in0=t2, scalar1=-0.5, scalar2=1.5,
                        op0=mybir.AluOpType.mult, op1=mybir.AluOpType.add)
                    nc.vector.tensor_tensor(y, y, t2, mybir.AluOpType.mult)
                return y

            for r in range(RB):
                # load x[r] as 6 natural tiles (c-part, din-free)
                xt = p_x.tile([D, NT, D], TDT, tag="xt")
                nc.sync.dma_start(
                    out=xt, in_=x_in[r].rearrange("(t p) d -> p t d", p=D)
                )
                # mask[r] broadcast to all 128 partitions (stride-0 DMA)
                mask_bc = p_msk.tile([D, N], TDT, tag="mbc")
                nc.sync.dma_start(
                    out=mask_bc,
                    in_=bass.AP(
                        tensor=mask_in.tensor,
                        offset=mask_in.offset + r * N,
                        ap=[[0, D], [1, N]],
                    ),
                )
                # LN stats per c-tile (bn_stats free-size cap is 512)
                st = p_stat.tile([D, NT, 6], F32, tag="st")
                mvs = p_stat.tile([D, NT, 2], F32, tag="mvs")
                for t in range(NT):
                    nc.vector.bn_stats(out=st[:, t, :], in_=xt[:, t, :])
                    nc.vector.bn_aggr(out=mvs[:, t, :], in_=st[:, t, :])
                rstds = newton_rsqrt(p_stat, mvs[:, :, 1], D)

                zt_slab = p_zt.tile([D, N], TDT, tag="zt")
                zpt4 = ps_t.tile([D, 512], TDT, tag="zpt4")
                zpt2 = ps_t.tile([D, 256], TDT, tag="zpt2")
                for t in range(NT):
                    # z = (x - mean) * rstd   (bf16 out)
                    z = p_tmp.tile([D, D], TDT, tag="z")
                    nc.vector.tensor_scalar(
                        out=z, in0=xt[:, t, :],
                        scalar1=mvs[:, t, 0:1], scalar2=rstds[:, t:t + 1],
                        op0=mybir.AluOpType.subtract, op1=mybir.AluOpType.mult,
                    )
                    # transpose z -> (din, c), packed 4+2 per psum bank
                    dst = zpt4[:, (t % 4) * D:(t % 4 + 1) * D] if t < 4 else \
                        zpt2[:, (t - 4) * D:(t - 3) * D]
                    nc.tensor.matmul(dst, z, ident, is_transpose=True,
                                     skip_group_check=True)
                    if t == 3:
                        nc.scalar.activation(
                            out=zt_slab[:, 0:512], in_=zpt4,
                            func=mybir.ActivationFunctionType.Copy)
                    if t == 5:
                        nc.scalar.activation(
                            out=zt_slab[:, 512:768], in_=zpt2,
                            func=mybir.ActivationFunctionType.Copy)

                # gated projection, two 384-wide chunks over c
                hh = p_hh.tile([D, N], TDT, tag="hh")
                CH = 384
                for ch in range(N // CH):
                    cs = slice(ch * CH, (ch + 1) * CH)
                    pp = ps_p.tile([D, CH], F32, tag="pp")
                    pg = ps_g.tile([D, CH], F32, tag="pg")
                    nc.tensor.matmul(pp, w1_sb[:, 0:D], zt_slab[:, cs],
                                     start=True, stop=not c1_nonzero)
                    nc.tensor.matmul(pg, w1_sb[:, D:2 * D], zt_slab[:, cs],
                                     start=True, stop=not c1_nonzero)
                    if c1_nonzero:
                        nc.tensor.matmul(pp, c1_sb[:, 0:D], ones_sb[:, :CH],
                                         start=False, stop=True,
                                         skip_group_check=True)
                        nc.tensor.matmul(pg, c1_sb[:, D:2 * D], ones_sb[:, :CH],
                                         start=False, stop=True,
                                         skip_group_check=True)
                    # s = sigmoid(g); s_m = s*mask; hh = p*s_m
                    s = p_tmp.tile([D, CH], TDT, tag="s")
                    nc.scalar.activation(
                        out=s, in_=pg, func=mybir.ActivationFunctionType.Sigmoid,
                    )
                    sm = p_tmp.tile([D, CH], TDT, tag="sm")
                    nc.vector.tensor_tensor(sm, s, mask_bc[:, cs],
                                            mybir.AluOpType.mult)
                    nc.vector.tensor_tensor(hh[:, cs], pp, sm,
                                            mybir.AluOpType.mult)

                # ship: partition block d*16.. holds dest d's channels
                for d in range(NC):
                    nc.sync.dma_start(
                        out=a2a1_in[d, r],
                        in_=hh[d * CPD:(d + 1) * CPD, :],
                    )

        # =========================================================
        # AllToAll #1
        # =========================================================
        nc.gpsimd.collective_compute(
            "AllToAll", mybir.AluOpType.bypass,
            replica_groups=rg,
            ins=[a2a1_in[:].opt()],
            outs=[a2a1_out[:].opt()],
        )

        # =========================================================
        # Phase E: einsums, channel-sharded.
        # a2a1_out[src, r, t', c]: global row = src*96+r.
        #   t' 0:4 a1, 4:8 b1 (need transpose to (c, row));
        #   t' 8:12 a2, 12:16 b2 (natural (row, c) is already (k, free)).
        # =========================================================
        with ExitStack() as ectx:
            e_nat = ectx.enter_context(tc.tile_pool(name="e_nat", bufs=2))
            e_op = ectx.enter_context(tc.tile_pool(name="e_op", bufs=2))
            e_st = ectx.enter_context(tc.tile_pool(name="e_st", bufs=3))
            ps_e = ectx.enter_context(
                tc.tile_pool(name="ps_e", bufs=3, space="PSUM"))
            ps_et = ectx.enter_context(
                tc.tile_pool(name="ps_et", bufs=2, space="PSUM"))

            def load_nat(tp):
                """Load channel tp as natural (row, c) slab (128, 6, 768)."""
                slab = e_nat.tile([D, NT, N], TDT, tag="nat")
                for src in range(NC):
                    r0 = src * RB  # global row of piece start
                    lo = r0
                    while lo < r0 + RB:
                        kt = lo // D
                        hi = min(r0 + RB, (kt + 1) * D)
                        nc.sync.dma_start(
                            out=slab[lo - kt * D: hi - kt * D, kt, :],
                            in_=a2a1_out[src, lo - r0: hi - r0, tp, :],
                        )
                        lo = hi
                return slab

            def transpose_slab(nat):
                """(row, c) natural slab -> (c, row) slab, both (128, 6, 768)."""
                tsl = e_op.tile([D, NT, N], TDT, tag="top")
                for rt in range(NT):
                    for ct in range(NT):
                        pt = ps_et.tile([D, D], TDT, tag="pt")
                        nc.tensor.transpose(
                            pt, nat[:, rt, ct * D:(ct + 1) * D], ident)
                        nc.vector.tensor_copy(
                            out=tsl[:, ct, rt * D:(rt + 1) * D], in_=pt)
                return tsl

            JC = 384
            for tt in range(2 * TPC):  # 4 x1-channels then 4 x2-channels
                if tt < TPC:
                    lhs_nat = load_nat(tt)          # a1 (i,k) natural
                    lhsT = transpose_slab(lhs_nat)  # (k, i)
                    rhs_nat = load_nat(TPC + tt)    # b1 (j,k) natural
                    rhsT = transpose_slab(rhs_nat)  # (k, j)
                else:
                    lhsT = load_nat(2 * TPC + (tt - TPC))  # a2 (k,i) natural
                    rhsT = load_nat(3 * TPC + (tt - TPC))  # b2 (k,j) natural
                for it in range(NT):
                    for jc in range(N // JC):
                        pe = ps_e.tile([D, JC], F32, tag="pe")
                        for kt in range(NT):
                            nc.tensor.matmul(
                                pe,
                                lhsT[:, kt, it * D:(it + 1) * D],
                                rhsT[:, kt, jc * JC:(jc + 1) * JC],
                                start=(kt == 0), stop=(kt == NT - 1),
                            )
                        st = e_st.tile([D, JC], TDT, tag="est")
                        nc.vector.tensor_copy(out=st, in_=pe)
                        # write to a2a2_in[dest, tt, i_local, j]
                        lo = it * D
                        while lo < (it + 1) * D:
                            dst = lo // RB
                            hi = min((it + 1) * D, (dst + 1) * RB)
                            nc.sync.dma_start(
                                out=a2a2_in[dst, tt, lo - dst * RB: hi - dst * RB,
                                            jc * JC:(jc + 1) * JC],
                                in_=st[lo - it * D: hi - it * D, :],
                            )
                            lo = hi

        # =========================================================
        # AllToAll #2
        # =========================================================
        nc.gpsimd.collective_compute(
            "AllToAll", mybir.AluOpType.bypass,
            replica_groups=rg,
            ins=[a2a2_in[:].opt()],
            outs=[a2a2_out[:].opt()],
        )

        # =========================================================
        # Phase O: out-LN + gated out-projection, row-sharded.
        # a2a2_out[src, tt, i, j] -> x12T (64ch, 768) per local row i.
        # =========================================================
        with ExitStack() as octx:
            o_xt = octx.enter_context(tc.tile_pool(name="o_xt", bufs=3))
            o_tmp = octx.enter_context(tc.tile_pool(name="o_tmp", bufs=4))
            o_stat = octx.enter_context(tc.tile_pool(name="o_stat", bufs=6))
            o_zt = octx.enter_context(tc.tile_pool(name="o_zt", bufs=2))
            o_out = octx.enter_context(tc.tile_pool(name="o_out", bufs=2))
            ps_oti = octx.enter_context(
                tc.tile_pool(name="ps_oti", bufs=2, space="PSUM"))
            ps_oto = octx.enter_context(
                tc.tile_pool(name="ps_oto", bufs=2, space="PSUM"))
            ps_o = octx.enter_context(
                tc.tile_pool(name="ps_o", bufs=2, space="PSUM"))

            for i in range(RB):
                x12t = o_xt.tile([DH, N], TDT, tag="x12t")
                nc.sync.dma_start(
                    out=x12t,
                    in_=a2a2_out[:, :, i, :].rearrange("s t j -> (s t) j"),
                )
                # transpose to natural (tok, ch), 6 tiles packed in one bank
                pnat = ps_oti.tile([D, NT, DH], TDT, tag="pnat")
                for t in range(NT):
                    nc.tensor.matmul(
                        pnat[:, t, :], x12t[:, t * D:(t + 1) * D],
                        ident[:DH, :DH], is_transpose=True,
                        skip_group_check=True)
                st = o_stat.tile([D, NT, 6], F32, tag="ost")
                mvs = o_stat.tile([D, NT, 2], F32, tag="omvs")
                for t in range(NT):
                    nc.vector.bn_stats(out=st[:, t, :], in_=pnat[:, t, :])
                    nc.vector.bn_aggr(out=mvs[:, t, :], in_=st[:, t, :])
                rstds = newton_rsqrt(o_stat, mvs[:, :, 1], D)

                z2t_slab = o_zt.tile([DH, N], TDT, tag="z2t")
                zot4 = ps_oto.tile([DH, 512], TDT, tag="zot4")
                zot2 = ps_oto.tile([DH, 256], TDT, tag="zot2")
                outsl = o_out.tile([D, NT, D], F32, tag="outsl")
                for t in range(NT):
                    z2 = o_tmp.tile([D, DH], TDT, tag="z2")
                    nc.vector.tensor_scalar(
                        out=z2, in0=pnat[:, t, :],
                        scalar1=mvs[:, t, 0:1], scalar2=rstds[:, t:t + 1],
                        op0=mybir.AluOpType.subtract, op1=mybir.AluOpType.mult,
                    )
                    # transpose back to (ch, tok), packed 4+2 per psum bank
                    dst = zot4[:, (t % 4) * D:(t % 4 + 1) * D] if t < 4 else \
                        zot2[:, (t - 4) * D:(t - 3) * D]
                    nc.tensor.matmul(dst, z2, ident, is_transpose=True,
                                     skip_group_check=True)
                    if t == 3:
                        nc.scalar.activation(
                            out=z2t_slab[:, 0:512], in_=zot4,
                            func=mybir.ActivationFunctionType.Copy)
                    if t == 5:
                        nc.scalar.activation(
                            out=z2t_slab[:, 512:768], in_=zot2,
                            func=mybir.ActivationFunctionType.Copy)

                for t in range(NT):
                    ts_ = slice(t * D, (t + 1) * D)
                    # gated projection: psum (128 tok, 256)
                    po = ps_o.tile([D, 2 * D], F32, tag="po")
                    nc.tensor.matmul(po, z2t_slab[:, ts_], w2_sb,
                                     start=True, stop=not c2_nonzero)
                    if c2_nonzero:
                        nc.tensor.matmul(po, ones_sb[:, 0:D], c2_sb,
                                         start=False, stop=True,
                                         skip_group_check=True)
                    s2 = o_tmp.tile([D, D], TDT, tag="s2")
                    nc.scalar.activation(
                        out=s2, in_=po[:, D:2 * D],
                        func=mybir.ActivationFunctionType.Sigmoid,
                    )
                    nc.vector.tensor_tensor(
                        outsl[:, t, :], po[:, 0:D], s2, mybir.AluOpType.mult
                    )
                nc.sync.dma_start(
                    out=out[i].rearrange("(t p) d -> p t d", p=D),
                    in_=outsl,
                )

    _split_multiwaits(nc)
    return nc


_BUILD_CACHE = {}


def kernel(x, mask, w_norm_in, b_norm_in, W_p_in, W_g_in,
           w_norm_out, b_norm_out, W_p_out, W_g_out):
    x = np.asarray(x, np.float32)
    mask = np.asarray(mask, np.float32)
    w_norm_in = np.asarray(w_norm_in, np.float32)
    b_norm_in = np.asarray(b_norm_in, np.float32)
    W_p_in = np.asarray(W_p_in, np.float32)
    W_g_in = np.asarray(W_g_in, np.float32)
    w_norm_out = np.asarray(w_norm_out, np.float32)
    b_norm_out = np.asarray(b_norm_out, np.float32)
    W_p_out = np.asarray(W_p_out, np.float32)
    W_g_out = np.asarray(W_g_out, np.float32)

    B = x.shape[0]
    assert x.shape == (B, N, N, D) and B == 1

    # ---- host-side weight folding & permutations ----
    W1p = w_norm_in[:, None] * W_p_in
    W1g = w_norm_in[:, None] * W_g_in
    c1 = np.concatenate([b_norm_in @ W_p_in, b_norm_in @ W_g_in])
    # dest-major output-channel permutation for phase P. hh channel c uses
    # p-col c and g-col c, so permute both the same way.
    perm = np.empty(D, np.int64)
    for d in range(NC):
        blk = d * CPD
        perm[blk:blk + 4] = np.arange(4) + 4 * d             # a1
        perm[blk + 4:blk + 8] = 32 + np.arange(4) + 4 * d    # b1
        perm[blk + 8:blk + 12] = 64 + np.arange(4) + 4 * d   # a2
        perm[blk + 12:blk + 16] = 96 + np.arange(4) + 4 * d  # b2
    W1 = np.concatenate([W1p[:, perm], W1g[:, perm]], axis=1)
    c1_perm = np.concatenate([c1[:D][perm], c1[D:][perm]])[None, :]

    # phase O: assembled channel q = src*8 + tt ; global channel:
    #   tt<4 -> 4*src+tt (x1 half), tt>=4 -> 32 + 4*src + (tt-4) (x2 half)
    qperm = np.empty(DH, np.int64)
    for src in range(NC):
        for t2 in range(8):
            g = 4 * src + t2 if t2 < 4 else 32 + 4 * src + (t2 - 4)
            qperm[src * 8 + t2] = g
    W2p = w_norm_out[:, None] * W_p_out
    W2g = w_norm_out[:, None] * W_g_out
    W2 = np.concatenate([W2p[qperm, :], W2g[qperm, :]], axis=1)
    c2 = np.concatenate([b_norm_out @ W_p_out, b_norm_out @ W_g_out])[None, :]

    c1_nz = bool(np.any(c1_perm != 0))
    c2_nz = bool(np.any(c2 != 0))

    key = (c1_nz, c2_nz)
    if key not in _BUILD_CACHE:
        _BUILD_CACHE[key] = build_kernel(c1_nz, c2_nz)
    nc_prog = _BUILD_CACHE[key]

    xs = x[0].astype(TDT_NP)                  # (768, 768, 128)
    w1_np = np.ascontiguousarray(W1.astype(TDT_NP))
    w2_np = np.ascontiguousarray(W2.astype(TDT_NP))
    in_maps = []
    for c in range(NC):
        in_maps.append({
            "x_rows": np.ascontiguousarray(xs[c * RB:(c + 1) * RB]),
            "mask_rows": np.ascontiguousarray(
                mask[0, c * RB:(c + 1) * RB].astype(TDT_NP)),
            "w1": w1_np,
            "w2": w2_np,
            "c1": c1_perm.astype(TDT_NP),
            "c2": c2.astype(TDT_NP),
        })

    res = bass_utils.run_bass_kernel_spmd(
        nc_prog, in_maps, core_ids=list(range(NC)),
        **getattr(kernel, "_run_kwargs", {}),
    )
    kernel._last_results = res
    outp = np.concatenate(
        [res.results[c]["out_rows"] for c in range(NC)], axis=0)
    return outp[None]  # (1, 768, 768, 128)


if __name__ == "__main__":
    import reference as R
    inputs = {k: np.asarray(v) for k, v in R.setup_inputs().items()}
    got = kernel(**inputs)
    exp = np.asarray(R.reference(**inputs))
    err = np.abs(got - exp)
    print("max abs err", err.max(), "absmax rel", err.max() / np.abs(exp).max())
